# revision 21
# baseline (speedup 1.0000x reference)
"""Trainium2 Bass kernel for DPAttention (attention block + residual + LayerNorm).

Sharding: 8 cores = DP2 (batch) x TP4 (head groups of 3 heads).
Core c: b = c//4, g = c%4 -> heads [3g, 3g+3).

Architecture (v2):
  - Q^T/K^T computed with heads 0,1 stacked on partitions 0:64/64:128 and head 2
    duplicated on both halves (via col-tiled concurrent matmuls).
  - Scores per (quarter u, key-tile kt): row-tiled concurrent 64-contraction
    matmuls fill one PSUM tile [128 keys, 3 heads, 256 q]; h2 splits its queries
    across the two row groups using the duplicate.
  - exp on ScalarE (table exp) or VectorE (fast-exp int16 bit trick), split by a
    static schedule to balance the two engines. Mask bias is -200 (not -1e9) so
    the bit-trick output stays in int16 range; masked lanes produce ~1e-8 noise.
  - ctx per head accumulates [V | 1] (65 rows) over 16 key tiles; row 64 gives
    the softmax denominator; a rank-1 (1e18 * mean_k V) matmul overwrites
    invalid-query columns.
  - Output dense is row-sharded over heads: each core computes a partial
    [512, 769] (769th col = Wo row-sums, gives the LayerNorm mean for free)
    per quarter, DMAs PSUM->DRAM, then a per-quarter ReduceScatter sums the
    4 partials and statically delivers 128 rows to each core. Residual +
    LayerNorm (rstd = exp(-0.5 ln var), no table switch) finish locally.
"""
import numpy as np
import ml_dtypes

import concourse.bass as bass
import concourse.mybir as mybir
import concourse.tile as tile
from concourse import bacc
from concourse.bass_utils import run_bass_kernel_spmd

F32 = mybir.dt.float32
BF16 = mybir.dt.bfloat16
I16 = mybir.dt.int16
AF = mybir.ActivationFunctionType
ALU = mybir.AluOpType

B, S, H, NH, HD = 2, 2048, 768, 12, 64
P = 128
KT = H // P            # 6 contraction tiles over hidden
ST = S // P            # 16 tiles over keys
TP = 4                 # head groups (tensor-parallel within a batch)
HG = NH // TP          # 3 heads per core
HGD = HG * HD          # 192
NQ = 4                 # query quarters
QS = S // NQ           # 512 queries per quarter
EPS = 1e-5
SCALE = 1.0 / np.sqrt(HD)
NCORES = 8
GROUPS = [[0, 1, 2, 3], [4, 5, 6, 7]]
MASKNEG = -200.0
BIGPOS = 1.0e18
HP1 = H + 1            # dense output cols + rowsum col

# fast-exp magic: int16 bits = y*A16*SCALE + (mask*A16 + B16), bitcast as bf16
A16 = 184.6650390625           # 2^23 / ln2 / 65536
B16 = (127.0 - 0.0435) * 128.0

# exp engine split: kts handled by DVE fast-exp (rest on ScalarE)
DVE_KTS = frozenset((2, 5, 8, 11, 14))

import os
STAGE = int(os.environ.get("KSTAGE", "4"))

_cache = {}


def build():
    nc = bacc.Bacc(num_devices=NCORES)

    xt_d = nc.dram_tensor("xt", [H, S], BF16, kind="ExternalInput")
    wq_d = nc.dram_tensor("wq", [H, HGD], BF16, kind="ExternalInput")
    wk_d = nc.dram_tensor("wk", [H, HGD], BF16, kind="ExternalInput")
    wv_d = nc.dram_tensor("wv", [H, HGD], BF16, kind="ExternalInput")
    bqk_d = nc.dram_tensor("bqk", [P, 4], F32, kind="ExternalInput")
    bvr_d = nc.dram_tensor("bvr", [P, HGD], F32, kind="ExternalInput")
    wo_d = nc.dram_tensor("wo", [P, 2, HP1], BF16, kind="ExternalInput")
    mkb_d = nc.dram_tensor("mkb", [S], F32, kind="ExternalInput")
    gq_d = nc.dram_tensor("gq", [1, S], BF16, kind="ExternalInput")
    xres_d = nc.dram_tensor("xres", [P, NQ, H], F32, kind="ExternalInput")
    xsum_d = nc.dram_tensor("xsum", [P, NQ], F32, kind="ExternalInput")
    lng_d = nc.dram_tensor("lng", [P, H], F32, kind="ExternalInput")
    lnb_d = nc.dram_tensor("lnb", [P, H], F32, kind="ExternalInput")
    out_d = nc.dram_tensor("out", [P, NQ, H], F32, kind="ExternalOutput")
    DEBUG = int(os.environ.get("KDEBUG", "0"))
    if DEBUG:
        dqt_d = nc.dram_tensor("dqt", [P, S], BF16, kind="ExternalOutput")
        dkp_d = nc.dram_tensor("dkp", [P, S], BF16, kind="ExternalOutput")
        de_d = nc.dram_tensor("de", [P, HG, QS], BF16, kind="ExternalOutput")
        dca_d = nc.dram_tensor("dca", [P, QS], BF16, kind="ExternalOutput")
        dcb_d = nc.dram_tensor("dcb", [HD, QS], BF16, kind="ExternalOutput")
        dden_d = nc.dram_tensor("dden", [1, HG * QS], F32, kind="ExternalOutput")
        dpart_d = nc.dram_tensor("dpart", [QS, HP1], F32, kind="ExternalOutput")
        drso_d = nc.dram_tensor("drso", [P, HP1], F32, kind="ExternalOutput")

    with tile.TileContext(nc) as tc:
        with (
            tc.tile_pool(name="wts", bufs=1) as wts,
            tc.tile_pool(name="qkv", bufs=1) as qkv,
            tc.tile_pool(name="dram", bufs=1, space="DRAM") as dram,
        ):
            # ---- activation table preload (Ln+Exp set), before any DMA ----
            dum = wts.tile([1, 2], F32)
            nc.gpsimd.memset(dum[:], 1.0)
            nc.scalar.activation(dum[:, 0:1], dum[:, 1:2], AF.Ln)
            nc.scalar.activation(dum[:, 1:2], dum[:, 0:1], AF.Exp)

            # ---- load weights / small tensors ----
            wo_sb = wts.tile([P, 2, HP1], BF16)
            nc.sync.dma_start(wo_sb[:], wo_d[:])

            bqk_sb = wts.tile([P, 4], F32)
            nc.gpsimd.dma_start(bqk_sb[:], bqk_d[:])
            bvr_sb = wts.tile([P, HG, HD], F32)
            nc.sync.dma_start(bvr_sb[:], bvr_d.rearrange("p (h d) -> p h d", d=HD))
            mkb_sb = wts.tile([P, ST], F32)
            nc.gpsimd.dma_start(mkb_sb[:], mkb_d.rearrange("(kt p) -> p kt", p=P))
            gq_sb = wts.tile([1, S], BF16)
            nc.gpsimd.dma_start(gq_sb[:], gq_d[:])
            lng_sb = wts.tile([P, H], F32)
            lnb_sb = wts.tile([P, H], F32)
            nc.sync.dma_start(lng_sb[:], lng_d[:])
            nc.sync.dma_start(lnb_sb[:], lnb_d[:])
            xres_sb = wts.tile([P, NQ, H], F32)
            nc.sync.dma_start(xres_sb[:], xres_d[:])
            xsum_sb = wts.tile([P, NQ], F32)
            nc.gpsimd.dma_start(xsum_sb[:], xsum_d[:])

            ones_sb = wts.tile([P, 1], BF16)
            nc.gpsimd.memset(ones_sb[:], 1.0)
            # DVE fast-exp per-partition bias: mkb*A16 + B16
            mkd_sb = wts.tile([P, ST], F32)
            nc.vector.tensor_scalar(mkd_sb[:], mkb_sb[:], float(A16), float(B16),
                                    op0=ALU.mult, op1=ALU.add)

            # ---- persistent intermediates ----
            qt_sb = qkv.tile([P, S], BF16)      # Q^T h0 rows 0:64, h1 rows 64:128
            qt2_sb = qkv.tile([P, S], BF16)     # Q^T h2 duplicated on both halves
            kp_sb = qkv.tile([P, S], BF16)      # K^T h0/h1 (free dim = keys)
            kp2_sb = qkv.tile([P, S], BF16)     # K^T h2 duplicated
            v_sb = qkv.tile([P, ST, HG, HD + 1], BF16)  # [V | 1] per key tile/head
            u_sb = qkv.tile([1, HG, HD + 1], BF16)      # mean_k V, 1.0 in slot HD
            nc.gpsimd.memset(v_sb[:, :, :, HD:HD + 1], 1.0)

            den3_sb = qkv.tile([1, HG * QS], F32, bufs=2)
            rden_dr = dram.tile([3, QS], F32, bufs=2)
            h_sb = qkv.tile([P, H], F32, bufs=2)
            mu_sb = qkv.tile([P, NQ], F32)
            s2_sb = qkv.tile([P, NQ], F32)
            var_sb = qkv.tile([P, 1], F32, bufs=2)
            rstd_sb = qkv.tile([P, 1], F32, bufs=2)
            nmr_sb = qkv.tile([P, 1], F32, bufs=2)
            sq_sb = qkv.tile([P, H], F32, bufs=2)
            t1_sb = qkv.tile([P, H], F32, bufs=2)
            o_sb = qkv.tile([P, H], F32, bufs=2)
            hps_sb = qkv.tile([P, HP1], F32, bufs=2)

            part_dr = dram.tile([NQ, QS, HP1], F32)
            rso_dr = dram.tile([NQ, P, HP1], F32)

            # ============ prologue: Q/K/V projections (xt scope) ============
            import contextlib
            with contextlib.ExitStack() as pro_ctx:
                xtp = pro_ctx.enter_context(tc.tile_pool(name="xt", bufs=1))
                pps = pro_ctx.enter_context(
                    tc.tile_pool(name="pps", bufs=3, space="PSUM"))
                xt_sb = xtp.tile([P, KT, S], BF16)
                xt_r = xt_d.rearrange("(kt p) s -> p kt s", p=P)
                for kt in range(KT):
                    nc.sync.dma_start(xt_sb[:, kt, :], xt_r[:, kt, :])
                wq_sb = xtp.tile([P, KT, HGD], BF16)
                wk_sb = xtp.tile([P, KT, HGD], BF16)
                wv_sb = xtp.tile([P, KT, HGD], BF16)
                nc.sync.dma_start(wq_sb[:], wq_d.rearrange("(kt p) d -> p kt d", p=P))
                nc.sync.dma_start(wk_sb[:], wk_d.rearrange("(kt p) d -> p kt d", p=P))
                nc.sync.dma_start(wv_sb[:], wv_d.rearrange("(kt p) d -> p kt d", p=P))

                def emit_proj(w_sb, dst01, dst2, bcol):
                    for qc in range(S // 512):
                        qs = slice(qc * 512, (qc + 1) * 512)
                        ps0 = pps.tile([P, 512], F32, tag="pj")
                        for kt in range(KT):
                            nc.tensor.matmul(
                                ps0[:], w_sb[:, kt, 0:P], xt_sb[:, kt, qs],
                                start=(kt == 0), stop=(kt == KT - 1))
                        nc.vector.tensor_scalar_add(
                            dst01[:, qs], ps0[:], bqk_sb[:, bcol:bcol + 1])
                        ps1 = pps.tile([P, 512], F32, tag="pj")
                        for kt in range(KT):
                            lhsT = w_sb[:, kt, P:HGD]
                            nc.tensor.matmul(
                                ps1[0:HD, :], lhsT, xt_sb[:, kt, qs],
                                start=(kt == 0), stop=(kt == KT - 1),
                                tile_position=(0, 0), skip_group_check=True)
                            nc.tensor.matmul(
                                ps1[HD:P, :], lhsT, xt_sb[:, kt, qs],
                                start=(kt == 0), stop=(kt == KT - 1),
                                tile_position=(0, 64), skip_group_check=True)
                        nc.vector.tensor_scalar_add(
                            dst2[:, qs], ps1[:], bqk_sb[:, bcol + 1:bcol + 2])

                emit_proj(wk_sb, kp_sb, kp2_sb, 2)
                emit_proj(wq_sb, qt_sb, qt2_sb, 0)

                # V projection (+ mean_k V) while xt is still resident
                for st in range(ST):
                    ps = pps.tile([P, 512], F32, tag="pj", name=f"vp{st}")
                    for kt in range(KT):
                        nc.tensor.matmul(
                            ps[:, 0:HGD], xt_sb[:, kt, st * P:(st + 1) * P],
                            wv_sb[:, kt, :],
                            start=(kt == 0), stop=(kt == KT - 1))
                    nc.vector.tensor_tensor(
                        v_sb[:, st, :, 0:HD],
                        ps[:, 0:HGD].rearrange("p (h d) -> p h d", d=HD),
                        bvr_sb[:], op=ALU.add)
                ps = pps.tile([P, 512], F32, tag="pj", name="ups")
                for st in range(ST):
                    nc.tensor.matmul(
                        ps[0:1, 0:HGD], ones_sb[:], v_sb[:, st, :, 0:HD],
                        start=(st == 0), stop=(st == ST - 1))
                nc.vector.tensor_scalar_mul(
                    u_sb[0:1, :, 0:HD],
                    ps[0:1, 0:HGD].rearrange("p (h d) -> p h d", d=HD),
                    1.0 / S)
                nc.gpsimd.memset(u_sb[:, :, HD:HD + 1], 1.0)

            # ============= attention (software-pipelined quarters) =========
            if True:
                attn_ctx = contextlib.ExitStack()
                epool = attn_ctx.enter_context(tc.tile_pool(name="epool", bufs=2))
                sps = attn_ctx.enter_context(
                    tc.tile_pool(name="sps", bufs=2, space="PSUM"))
                cps = attn_ctx.enter_context(
                    tc.tile_pool(name="cps", bufs=1, space="PSUM"))
                dps = attn_ctx.enter_context(
                    tc.tile_pool(name="dps", bufs=1, space="PSUM"))
                npool = attn_ctx.enter_context(tc.tile_pool(name="npool", bufs=2))

                e_tiles = {}

                def emit_scores_kt(u, kt):
                    e_t = e_tiles[u]
                    q0 = u * QS
                    qs = slice(q0, q0 + QS)
                    kts = slice(kt * P, (kt + 1) * P)
                    # one PSUM bank per concurrent row-group matmul
                    ps = sps.tile([P, HG, QS], F32, tag="s", name=f"sc{u}_{kt}")
                    nc.tensor.matmul(ps[:, 0, :], kp_sb[0:HD, kts],
                                     qt_sb[0:HD, qs],
                                     start=True, stop=True, tile_position=(0, 0))
                    nc.tensor.matmul(ps[:, 1, :], kp_sb[HD:P, kts],
                                     qt_sb[HD:P, qs],
                                     start=True, stop=True, tile_position=(64, 0))
                    nc.tensor.matmul(ps[:, 2, :], kp2_sb[0:HD, kts],
                                     qt2_sb[0:HD, qs],
                                     start=True, stop=True, tile_position=(0, 0))

                    def ex_act(dst, src, k=kt):
                        nc.scalar.activation(dst, src, AF.Exp,
                                             bias=mkb_sb[:, k:k + 1],
                                             scale=float(SCALE))

                    def ex_dve(dst, src, k=kt):
                        nc.vector.tensor_scalar(
                            dst.bitcast(I16), src,
                            scalar1=float(SCALE * A16),
                            scalar2=mkd_sb[:, k:k + 1],
                            op0=ALU.mult, op1=ALU.add)

                    ab_dve = kt in DVE_KTS
                    (ex_dve if ab_dve else ex_act)(
                        e_t[:, kt, 0:2, :], ps[:, 0:2, :])
                    (ex_act if ab_dve else ex_dve)(
                        e_t[:, kt, 2, :], ps[:, 2, :])

                def ctx_ops(u):
                    """List of closures: ctx MMs + per-head tails + dense +
                    RS + LN for quarter u, to dribble between score kts."""
                    e_t = e_tiles[u]
                    q0 = u * QS
                    state = {}
                    ops = []

                    def mk_mm(h, kt):
                        def op():
                            if "pc" not in state or state["pch"] != h:
                                state["pc"] = cps.tile([HD + 1, QS], F32, tag="c",
                                                       name=f"c{u}_{h}")
                                state["pch"] = h
                            nc.tensor.matmul(
                                state["pc"][:], v_sb[:, kt, h, :],
                                e_t[:, kt, h, :], start=(kt == 0), stop=False)
                        return op

                    def mk_tail(h):
                        def op():
                            pc = state["pc"]
                            nc.tensor.matmul(
                                pc[:], u_sb[0:1, h, :], gq_sb[0:1, q0:q0 + QS],
                                start=False, stop=True)
                            den = den3_sb[0:1, h * QS:(h + 1) * QS]
                            nc.vector.reciprocal(den, pc[HD:HD + 1, :])
                            nc.sync.dma_start(rden_dr[h:h + 1, :], den)
                            if h == 0:
                                state["rb"] = npool.tile([HD, HG, QS], F32,
                                                         tag="rb", name=f"rb{u}")
                                state["ctxa"] = npool.tile([P, QS], BF16,
                                                           tag="ca", name=f"ca{u}")
                                state["ctxb"] = npool.tile([HD, QS], BF16,
                                                           tag="cb", name=f"cb{u}")
                            rb = state["rb"]
                            nc.sync.dma_start(
                                rb[:, h, :],
                                rden_dr[h:h + 1, :].to_broadcast((HD, QS)))
                            dst = (state["ctxb"][:] if h == 2
                                   else state["ctxa"][h * HD:(h + 1) * HD, :])
                            nc.vector.tensor_tensor(dst, pc[0:HD, :], rb[:, h, :],
                                                    op=ALU.mult)
                        return op

                    for h in range(HG):
                        for kt in range(ST):
                            ops.append(mk_mm(h, kt))
                        ops.append(mk_tail(h))

                    if STAGE >= 3:
                        def mk_dense(st, c0, c1):
                            def op():
                                qs = slice(st * P, (st + 1) * P)
                                ps = dps.tile([P, 512], F32, tag="d",
                                              name=f"de{u}_{st}_{c0}")
                                nc.tensor.matmul(ps[:, 0:c1 - c0],
                                                 state["ctxa"][:, qs],
                                                 wo_sb[:, 0, c0:c1],
                                                 start=True, stop=False)
                                nc.tensor.matmul(ps[:, 0:c1 - c0],
                                                 state["ctxb"][:, qs],
                                                 wo_sb[0:HD, 1, c0:c1],
                                                 start=False, stop=True)
                                dcp = npool.tile([P, 512], F32, tag="dcp",
                                                  name=f"dcp{u}_{st}_{c0}")
                                if c0 == 0:
                                    nc.scalar.activation(dcp[:, 0:c1 - c0],
                                                         ps[:, 0:c1 - c0],
                                                         AF.Identity)
                                else:
                                    nc.vector.tensor_copy(dcp[:, 0:c1 - c0],
                                                          ps[:, 0:c1 - c0])
                                nc.sync.dma_start(
                                    part_dr[u, st * P:(st + 1) * P, c0:c1],
                                    dcp[:, 0:c1 - c0])
                            return op
                        for st in range(QS // P):
                            for c0, c1 in ((0, 512), (512, HP1)):
                                ops.append(mk_dense(st, c0, c1))

                        def op_rs():
                            nc.gpsimd.collective_compute(
                                "ReduceScatter", ALU.add, replica_groups=GROUPS,
                                ins=[part_dr[u].opt()], outs=[rso_dr[u].opt()])
                        ops.append(op_rs)
                    if STAGE >= 4:
                        ops.append(lambda: emit_ln(u))
                    if DEBUG and u == 0:
                        def op_dbg():
                            nc.sync.dma_start(dca_d[:], state["ctxa"][:])
                            nc.sync.dma_start(dcb_d[:], state["ctxb"][:])
                            nc.sync.dma_start(dden_d[:], den3_sb[:])
                            nc.sync.dma_start(dpart_d[:], part_dr[0])
                            nc.sync.dma_start(drso_d[:], rso_dr[0])
                        ops.append(op_dbg)
                    return ops

                def emit_ln(u):
                    nc.sync.dma_start(hps_sb[:], rso_dr[u])
                    nc.vector.tensor_tensor(
                        h_sb[:], hps_sb[:, 0:H], xres_sb[:, u, :], op=ALU.add)
                    # mu = (dense_rowsum + xres_rowsum) / H
                    nc.vector.tensor_tensor(
                        mu_sb[:, u:u + 1], hps_sb[:, H:HP1],
                        xsum_sb[:, u:u + 1], op=ALU.add)
                    nc.vector.tensor_scalar_mul(
                        mu_sb[:, u:u + 1], mu_sb[:, u:u + 1], 1.0 / H)
                    # s2 = sum(h^2)/H ; var = s2 - mu^2 + EPS
                    nc.scalar.activation(sq_sb[:], h_sb[:], AF.Square,
                                         accum_out=s2_sb[:, u:u + 1])
                    nc.vector.tensor_scalar_mul(
                        s2_sb[:, u:u + 1], s2_sb[:, u:u + 1], 1.0 / H)
                    nc.vector.tensor_scalar(
                        var_sb[:], mu_sb[:, u:u + 1],
                        scalar1=mu_sb[:, u:u + 1], scalar2=-EPS,
                        op0=ALU.mult, op1=ALU.subtract)
                    nc.vector.tensor_tensor(
                        var_sb[:], s2_sb[:, u:u + 1], var_sb[:], op=ALU.subtract)
                    # rstd = exp(-0.5 ln var); nmr = -mu * rstd
                    nc.scalar.activation(rstd_sb[:], var_sb[:], AF.Ln)
                    nc.scalar.activation(rstd_sb[:], rstd_sb[:], AF.Exp, scale=-0.5)
                    nc.vector.tensor_scalar_mul(nmr_sb[:], rstd_sb[:], -1.0)
                    nc.vector.tensor_tensor(nmr_sb[:], nmr_sb[:],
                                            mu_sb[:, u:u + 1], op=ALU.mult)
                    # out = ((h*rstd + nmr) * gamma) + beta
                    nc.scalar.activation(t1_sb[:], h_sb[:], AF.Identity,
                                         bias=nmr_sb[:, 0:1],
                                         scale=rstd_sb[:, 0:1])
                    nc.vector.tensor_tensor(o_sb[:], t1_sb[:], lng_sb[:],
                                            op=ALU.mult)
                    nc.vector.tensor_tensor(o_sb[:], o_sb[:], lnb_sb[:],
                                            op=ALU.add)
                    nc.sync.dma_start(out_d[:, u, :], o_sb[:])

                for i in range(NQ + 1):
                    if i < NQ:
                        e_tiles[i] = epool.tile([P, ST, HG, QS], BF16, tag="e",
                                                name=f"e{i}")
                    ops = ctx_ops(i - 1) if (i > 0 and STAGE >= 2) else []
                    done = 0
                    for kt in range(ST):
                        if i < NQ:
                            emit_scores_kt(i, kt)
                        want = (len(ops) * (kt + 1)) // ST
                        while done < want:
                            ops[done]()
                            done += 1
                    while done < len(ops):
                        ops[done]()
                        done += 1
                    if i == 1 and DEBUG:
                        nc.sync.dma_start(de_d[:], e_tiles[0][:, 3, :, :])
                    if i > 0:
                        del e_tiles[i - 1]
                if DEBUG:
                    nc.sync.dma_start(dqt_d[:], qt_sb[:])
                    nc.sync.dma_start(dkp_d[:], kp_sb[:])
                if STAGE < 4:
                    for u in range(NQ):
                        nc.gpsimd.memset(o_sb[:], 0.0)
                        nc.sync.dma_start(out_d[:, u, :], o_sb[:])
                attn_ctx.close()

    nc.compile()
    return nc


def _prep_inputs(inputs):
    hs = np.asarray(inputs["hidden_states"], dtype=np.float32)
    am = np.asarray(inputs["attention_mask"], dtype=np.float32)
    Wq = np.asarray(inputs["Wq"], dtype=np.float32)
    Wk = np.asarray(inputs["Wk"], dtype=np.float32)
    Wv = np.asarray(inputs["Wv"], dtype=np.float32)
    Wo = np.asarray(inputs["Wo"], dtype=np.float32)
    bq = np.asarray(inputs["bq"], dtype=np.float32)
    bk = np.asarray(inputs["bk"], dtype=np.float32)
    bv = np.asarray(inputs["bv"], dtype=np.float32)
    bo = np.asarray(inputs["bo"], dtype=np.float32)
    lng = np.asarray(inputs["ln_gamma"], dtype=np.float32)
    lnb = np.asarray(inputs["ln_beta"], dtype=np.float32)

    lng_rep = np.ascontiguousarray(np.broadcast_to(lng, (P, H)))
    lnb_rep = np.ascontiguousarray(np.broadcast_to(lnb, (P, H)))

    in_maps = []
    for c in range(NCORES):
        b, g = c // TP, c % TP
        cs = slice(HGD * g, HGD * (g + 1))
        mk = np.where(am[b] >= 0, 0.0, MASKNEG).astype(np.float32)
        gqv = np.where(am[b] >= 0, 0.0, BIGPOS).astype(ml_dtypes.bfloat16)[None, :]
        # bqk: cols 0/1 = bq (h0h1 stacked / h2 duplicated), cols 2/3 = bk
        bqk = np.zeros((P, 4), dtype=np.float32)
        bqk[:, 0] = bq[cs][0:P]
        bqk[:, 1] = np.tile(bq[cs][P:HGD], 2)
        bqk[:, 2] = bk[cs][0:P]
        bqk[:, 3] = np.tile(bk[cs][P:HGD], 2)
        # wo: own 192 rows split 128 + 64(padded), with rowsum col appended
        wos = Wo[cs]  # [192, H]
        wo = np.zeros((P, 2, HP1), dtype=np.float32)
        wo[:, 0, 0:H] = wos[0:P]
        wo[0:HD, 1, 0:H] = wos[P:HGD]
        wo[:, 0, H] = wos[0:P].sum(axis=1)
        wo[0:HD, 1, H] = wos[P:HGD].sum(axis=1)
        # residual rows: for quarter u, rows u*512 + g*128 + [0:128)
        xres = np.zeros((P, NQ, H), dtype=np.float32)
        for u in range(NQ):
            r0 = u * QS + g * P
            xres[:, u, :] = hs[b, r0:r0 + P] + bo
        in_maps.append({
            "xt": np.ascontiguousarray(hs[b].T).astype(ml_dtypes.bfloat16),
            "wq": np.ascontiguousarray(Wq[:, cs]).astype(ml_dtypes.bfloat16),
            "wk": np.ascontiguousarray(Wk[:, cs]).astype(ml_dtypes.bfloat16),
            "wv": np.ascontiguousarray(Wv[:, cs]).astype(ml_dtypes.bfloat16),
            "bqk": bqk,
            "bvr": np.ascontiguousarray(np.broadcast_to(bv[cs], (P, HGD))),
            "wo": wo.astype(ml_dtypes.bfloat16),
            "mkb": mk,
            "gq": np.ascontiguousarray(gqv),
            "xres": xres,
            "xsum": np.ascontiguousarray(xres.sum(axis=2)),
            "lng": lng_rep,
            "lnb": lnb_rep,
        })
    return in_maps


def _run(inputs, trace=False, trace_cores=None):
    if "nc" not in _cache:
        _cache["nc"] = build()
    nc = _cache["nc"]
    in_maps = _prep_inputs(inputs)
    res = run_bass_kernel_spmd(
        nc, in_maps, list(range(NCORES)), trace=trace,
        trace_cores=trace_cores,
    )
    out = np.empty((B, S, H), dtype=np.float32)
    for c in range(NCORES):
        b, g = c // TP, c % TP
        for u in range(NQ):
            r0 = u * QS + g * P
            out[b, r0:r0 + P] = res.results[c]["out"][:, u, :]
    return out, res


def kernel(**inputs) -> np.ndarray:
    out, _ = _run(inputs)
    return out
